# revision 21
# baseline (speedup 1.0000x reference)
"""Multi-head attention (B=2, S=2048, D=1024, H=16) on 8 TRN2 NeuronCores.

Sharding: data parallel on batch (2) x tensor parallel on heads (4 groups of
4 heads).  Core c handles batch c//4, heads 4*(c%4) .. 4*(c%4)+4.  Each core
computes q/k/v projections for its 256 output dims, attention for its 4
heads, and a partial (row-parallel) output projection.  The host sums the 4
partials per batch and adds b_o.

Per-core kernel (all matmuls bf16 inputs, fp32 PSUM):
  - qT/kT d-major [256, 2048]; v s-major with a ones column per head
    zero-padded to 128 columns (full-array PV keeps the PE HAM clock
    un-throttled).  Even heads keep v at dd 0:64 with ones at 64 (PV out
    rows 0:64 data, 64 den); odd heads put v at dd 64:128 with ones at 63
    (PV out rows 64:128 data, 63 den) so the softmax normalization can
    write aoT in place on the Vector engine - no partition-shift DMA
    (den rows 64/32: partition bases must be 32-aligned).
  - scores are computed transposed (S[j, i] = k_j . q_i): no transposes
    anywhere.  The two heads of a pair run as K=64 matmuls on distinct PE
    row-groups (base partitions 0/64) writing the two halves of one shared
    PSUM tile - they execute concurrently, so the array stays fully busy and
    the score cost halves vs zero-padding.
  - softmax exp runs on the Scalar engine straight out of PSUM (one
    activation covers both heads); no max-subtraction (scores std ~0.33).
  - i-chunk 512: S-pair tile [128,1024] double-buffered (4 banks) + three
    [128,512] O accumulators (3) + one filler bank = exactly 8 PSUM banks.
  - projections / output-projection groups are emitted as fillers inside the
    attention steps so the PE works while the Scalar engine streams exps.
  - x is staged host-side as [p, c, kt*512] so each 512-col chunk loads with
    one (c>0) or eight (c=0, per-kt for progressive consumption) DMAs; w_k
    and w_q load in m-halves so the first matmul's weights land early.
  - a short burst of dummy matmuls on zeroed SBUF runs during the DMA
    lead-in so the PE activity manager ramps the clock before real work.
  - the last four output tiles (st 12-15) run phase-major after the last
    attention chunk: all kt2=0 partial matmuls issue first (they overlap the
    final softmax-normalization chain), then kt2=1, into 8 PSUM banks;
    evictions split across Scalar and Vector; one output DMA per tile.
"""

import os

import numpy as np
import ml_dtypes

B, S, D = 2, 2048, 1024
H, DH = 16, 64
N_CORES = 8
HPC = 4  # heads per core
DL = HPC * DH  # 256 local dims per core
KT = D // 128  # 8 k-tiles
ST = S // 128  # 16 s-tiles (also j-tiles)
IC = 512  # i-chunk (query chunk)
NIC = S // IC

_BF16 = ml_dtypes.bfloat16

_nc_cache = None


def _build_nc():
    from contextlib import ExitStack

    import concourse.mybir as mybir
    import concourse.tile as tile
    from concourse import bacc

    f32 = mybir.dt.float32
    bf16 = mybir.dt.bfloat16
    Alu = mybir.AluOpType
    Act = mybir.ActivationFunctionType

    nc = bacc.Bacc("TRN2", target_bir_lowering=False, debug=False, enable_asserts=False)

    xt_d = nc.dram_tensor("xt", (128, NIC, KT, 512), bf16, kind="ExternalInput")
    wq_d = nc.dram_tensor("wq", (128, 2, KT, 128), bf16, kind="ExternalInput")
    wk_d = nc.dram_tensor("wk", (128, 2, KT, 128), bf16, kind="ExternalInput")
    wv_d = nc.dram_tensor("wv", (128, KT, DL), bf16, kind="ExternalInput")
    wo_d = nc.dram_tensor("wo", (128, 2, D), bf16, kind="ExternalInput")
    bqk_d = nc.dram_tensor("bqk", (128, 4), f32, kind="ExternalInput")
    bv_d = nc.dram_tensor("bv", (128, DL), f32, kind="ExternalInput")
    out_d = nc.dram_tensor("out", (S, D), f32, kind="ExternalOutput")

    with tile.TileContext(nc) as tc, ExitStack() as ctx:
        consts = ctx.enter_context(tc.tile_pool(name="consts", bufs=1))
        xbf = consts.tile([128, NIC, KT, 512], bf16)  # [p, c, kt, s-in-chunk]
        wq_sb = consts.tile([128, 2, KT, 128], bf16)  # [p, mhalf, kt, m]
        wk_sb = consts.tile([128, 2, KT, 128], bf16)
        wv_sb = consts.tile([128, KT, DL], bf16)
        wo_sb = consts.tile([128, 2, D], bf16)  # [p, kt2, o]
        bqk_sb = consts.tile([128, 4], f32)
        bv_sb = consts.tile([128, DL], f32)
        qT = consts.tile([128, 2, S], bf16)  # [p, mt, s]
        kT = consts.tile([128, 2, S], bf16)
        # v (s-major), ones column per head, zero-padded to 128 cols.
        # even h: v at 0:64, ones at 64; odd h: ones at 0, v at 64:128.
        vaug = consts.tile([128, ST, HPC, 128], bf16)  # [p(j), jt, h, dd]
        aoT = consts.tile([128, 2, S], bf16)  # attn-out transposed [p, kt2, s]
        dum = consts.tile([128, 512], bf16)  # warm-up matmul operand

        # ---- input DMAs: everything is host-pre-tiled to [p, ...] layouts
        # so each load is one DMA with fat (multi-KB) contiguous descriptors.
        # wk m-half 0 + x chunk 0 gate the first matmul.
        nc.sync.dma_start(wk_sb[:, 0], wk_d.ap()[:, 0])
        nc.sync.dma_start(xbf[:, 0], xt_d.ap()[:, 0])
        nc.sync.dma_start(wq_sb[:, 0], wq_d.ap()[:, 0])
        nc.sync.dma_start(bqk_sb[:], bqk_d.ap())
        nc.sync.dma_start(wv_sb[:], wv_d.ap())
        nc.sync.dma_start(bv_sb[:], bv_d.ap())
        nc.sync.dma_start(xbf[:, 1], xt_d.ap()[:, 1])
        nc.sync.dma_start(wk_sb[:, 1], wk_d.ap()[:, 1])
        nc.sync.dma_start(wq_sb[:, 1], wq_d.ap()[:, 1])
        nc.sync.dma_start(xbf[:, 2], xt_d.ap()[:, 2])
        nc.sync.dma_start(xbf[:, 3], xt_d.ap()[:, 3])
        nc.sync.dma_start(wo_sb[:], wo_d.ap())

        nc.gpsimd.memset(dum[:], 0.0)
        for h in range(HPC):
            if h % 2 == 0:
                nc.gpsimd.memset(vaug[:, :, h, DH + 1 :], 0.0)
                nc.gpsimd.memset(vaug[:, :, h, DH : DH + 1], 1.0)
            else:
                nc.gpsimd.memset(vaug[:, :, h, 0:1], 1.0)
                nc.gpsimd.memset(vaug[:, :, h, 1:DH], 0.0)

        # Preload the exp activation table set (~2.7us) during the DMA
        # lead-in so the first real softmax exp doesn't pay for it.
        warm = consts.tile([128, 8], f32)
        nc.gpsimd.memset(warm[:], 0.0)
        nc.scalar.activation(warm[:], warm[:], Act.Exp)

        ps = ctx.enter_context(tc.tile_pool(name="ps", bufs=2, space="PSUM"))
        op_ = ctx.enter_context(tc.tile_pool(name="op", bufs=3, space="PSUM"))
        fp = ctx.enter_context(tc.tile_pool(name="fp", bufs=1, space="PSUM"))
        ep = ctx.enter_context(tc.tile_pool(name="ep", bufs=8))
        rp = ctx.enter_context(tc.tile_pool(name="rp", bufs=3))
        tp = ctx.enter_context(tc.tile_pool(name="tp", bufs=3))
        osb = ctx.enter_context(tc.tile_pool(name="osb", bufs=3))
        tlo = ctx.enter_context(tc.tile_pool(name="tlo", bufs=4))

        def qk_proj(proj, mt, c, alt=False, kts=None):
            """q (proj=0) / k (proj=1) projection, one 512-col chunk."""
            w_sb = wq_sb if proj == 0 else wk_sb
            dst_all = qT if proj == 0 else kT
            pool, tg = (op_, "O") if alt else (fp, "f")
            p = pool.tile([128, 512], f32, tag=tg)
            for kt in kts if kts is not None else range(KT):
                nc.tensor.matmul(
                    p[:],
                    w_sb[:, mt, kt, :],
                    xbf[:, c, kt, :],
                    start=(kt == 0),
                    stop=(kt == KT - 1),
                )
            dst = dst_all[:, mt, c * 512 : (c + 1) * 512]
            bias_ap = bqk_sb[:, proj * 2 + mt : proj * 2 + mt + 1]
            if proj == 0:
                nc.vector.tensor_scalar(dst, p[:], bias_ap, 0.125, Alu.add, Alu.mult)
            else:
                nc.vector.tensor_scalar(dst, p[:], bias_ap, None, Alu.add)

        def v_proj(st):
            pool = fp if st % 2 == 0 else op_
            p = pool.tile([128, DL], f32, tag="f" if st % 2 == 0 else "O")
            for kt in range(KT):
                nc.tensor.matmul(
                    p[:],
                    xbf[:, st // 4, kt, (st % 4) * 128 : (st % 4 + 1) * 128],
                    wv_sb[:, kt, :],
                    start=(kt == 0),
                    stop=(kt == KT - 1),
                )
            for h in range(HPC):
                dd = 0 if h % 2 == 0 else DH
                nc.vector.tensor_tensor(
                    vaug[:, st, h, dd : dd + DH],
                    p[:, h * DH : (h + 1) * DH],
                    bv_sb[:, h * DH : (h + 1) * DH],
                    Alu.add,
                )

        def o_proj_chunk(st, oc):
            pso = fp.tile([128, 512], f32, tag="f")
            for kt2 in range(2):
                nc.tensor.matmul(
                    pso[:],
                    aoT[:, kt2, st * 128 : (st + 1) * 128],
                    wo_sb[:, kt2, oc * 512 : (oc + 1) * 512],
                    start=(kt2 == 0),
                    stop=(kt2 == 1),
                )
            stg = osb.tile([128, 512], f32, tag="oh")
            nc.vector.tensor_copy(stg[:], pso[:])
            nc.sync.dma_start(
                out_d.ap()[st * 128 : (st + 1) * 128, oc * 512 : (oc + 1) * 512],
                stg[:],
            )

        def attn_norm(h, ic, O, direct=False):
            # even h: data rows 0:64, den row 64; odd h: data rows 64:128,
            # den row 0.  direct=True: the normalize multiply writes aoT in
            # place (same partition base) - used for the last chunk so the
            # tail o-proj starts sooner.  Otherwise stage via tmp + DMA (the
            # scheduler pipelines that better against o-proj fillers).
            pb, mt = 64 * (h % 2), h // 2
            den_row = DH if h % 2 == 0 else 0
            den = rp.tile([1, IC], f32, tag="den")
            if direct and h % 2 == 1:
                # Scalar engine copies head B's denominator so it doesn't
                # queue behind head A's chain on the Vector engine.
                nc.scalar.activation(den[:], O[den_row : den_row + 1, :], Act.Copy)
            else:
                nc.vector.tensor_copy(den[:], O[den_row : den_row + 1, :])
            recip = rp.tile([1, IC], f32, tag="r")
            nc.vector.reciprocal_approx_fast(recip[:], den[:])
            rb = rp.tile([64, IC], f32, tag="rb")
            nc.gpsimd.partition_broadcast(rb[:], recip[:])
            if direct:
                nc.vector.tensor_tensor(
                    aoT[pb : pb + 64, mt, ic * IC : (ic + 1) * IC],
                    O[pb : pb + 64, :],
                    rb[:],
                    Alu.mult,
                )
            else:
                tmp = tp.tile([64, IC], bf16, tag="t")
                nc.vector.tensor_tensor(tmp[:], O[pb : pb + 64, :], rb[:], Alu.mult)
                nc.sync.dma_start(
                    aoT[pb : pb + 64, mt, ic * IC : (ic + 1) * IC], tmp[:]
                )

        def pair_ic(pair, ic, fillers, mid_fillers={}, direct_norm=False):
            """Attention for head pair (2*pair, 2*pair+1) on query chunk ic.
            fillers: {jt: [callable, ...]} emitted just before that step;
            mid_fillers: emitted between the step's exp and its PV.
            The PV matmuls run one jt behind the scores/exp so the PE
            streams scores(jt+1) while exp(jt) is still on the Scalar
            engine instead of blocking in-order on PV(jt)."""
            hA, hB = 2 * pair, 2 * pair + 1
            OA = op_.tile([128, IC], f32, tag="O")
            OB = op_.tile([128, IC], f32, tag="O")
            prevE = None

            def pv(jt, E):
                nc.tensor.matmul(
                    OA[:],
                    vaug[:, jt, hA, :],
                    E[:, 0:IC],
                    start=(jt == 0),
                    stop=(jt == ST - 1),
                )
                nc.tensor.matmul(
                    OB[:],
                    vaug[:, jt, hB, :],
                    E[:, IC : 2 * IC],
                    start=(jt == 0),
                    stop=(jt == ST - 1),
                )

            for jt in range(ST):
                for f in fillers.get(jt, ()):
                    f()
                Sp = ps.tile([128, 2 * IC], f32, tag="S")
                nc.tensor.matmul(
                    Sp[:, 0:IC],
                    kT[0:64, pair, jt * 128 : (jt + 1) * 128],
                    qT[0:64, pair, ic * IC : (ic + 1) * IC],
                    start=True,
                    stop=True,
                )
                nc.tensor.matmul(
                    Sp[:, IC : 2 * IC],
                    kT[64:128, pair, jt * 128 : (jt + 1) * 128],
                    qT[64:128, pair, ic * IC : (ic + 1) * IC],
                    start=True,
                    stop=True,
                )
                E = ep.tile([128, 2 * IC], bf16, tag="E")
                nc.scalar.activation(E[:], Sp[:], Act.Exp)
                for f in mid_fillers.get(jt, ()):
                    f()
                if prevE is not None:
                    pv(jt - 1, prevE)
                prevE = E
            pv(ST - 1, prevE)
            attn_norm(hA, ic, OA, direct=direct_norm)
            attn_norm(hB, ic, OB, direct=direct_norm)

        # ---- emission schedule ----
        qk_proj(1, 0, 0)
        qk_proj(0, 0, 0, alt=True)
        v_proj(0)
        v_proj(1)
        F = lambda *fs: list(fs)
        p0i0 = {jt: F(lambda st=jt + 2: v_proj(st)) for jt in range(ST - 2)}
        for jt, c in ((2, 1), (5, 2), (9, 3)):
            p0i0[jt] = [lambda c=c: qk_proj(1, 0, c)] + p0i0[jt]
        p0i0[12] = [lambda: qk_proj(0, 0, 1)] + p0i0[12]
        pair_ic(0, 0, p0i0)
        pair_ic(0, 1, {
            4: F(lambda: qk_proj(1, 1, 0)),
            6: F(lambda: qk_proj(1, 1, 1)),
            8: F(lambda: qk_proj(1, 1, 2)),
            10: F(lambda: qk_proj(1, 1, 3)),
            13: F(lambda: qk_proj(0, 1, 0)),
        })
        pair_ic(1, 0, {
            4: F(lambda: qk_proj(0, 1, 1)),
            9: F(lambda: qk_proj(0, 0, 2)),
        })
        pair_ic(1, 1, {
            4: F(lambda: qk_proj(0, 1, 2)),
            9: F(lambda: qk_proj(0, 0, 3)),
            12: F(lambda: qk_proj(0, 1, 3)),
        })
        pair_ic(0, 2, {5 + i: F(lambda st=(i + 2) // 2, oc=i % 2: o_proj_chunk(st, oc))
                       for i in range(6)})
        pair_ic(1, 2, {4 + i: F(lambda st=4 + i // 2, oc=i % 2: o_proj_chunk(st, oc))
                       for i in range(8)})
        pair_ic(0, 3, {4 + i: F(lambda st=8 + i // 2, oc=i % 2: o_proj_chunk(st, oc))
                       for i in range(4)})
        pair_ic(1, 3, {4: F(lambda: o_proj_chunk(0, 0)),
                       8: F(lambda: o_proj_chunk(0, 1))}, direct_norm=True)

        # ---- dense tail: st 10-15 across all 8 PSUM banks, ordered by when
        # each bank frees.  st10/11 (whose aoT inputs are long ready) and the
        # st12 kt2=0 partials run while the final softmax-normalization chain
        # is on Vector/GpSimd; the ps banks are then re-used for st14/15.
        # st13 sits in the two banks the last pair's O accumulators vacate at
        # the norm multiplies.  kt2=1 for st12-15 follows the norm.
        def omm(piece, st, kt2, oc):
            nc.tensor.matmul(
                piece,
                aoT[:, kt2, st * 128 : (st + 1) * 128],
                wo_sb[:, kt2, oc * 512 : (oc + 1) * 512],
                start=(kt2 == 0),
                stop=(kt2 == 1),
            )

        def evict_dma(st, tile_full, engine):
            stg = tlo.tile([128, D], f32, tag="to")
            if engine == "scalar":
                nc.scalar.activation(stg[:], tile_full[:], Act.Copy)
            else:
                nc.vector.tensor_copy(stg[:, 0:512], tile_full[:, 0:512])
                nc.vector.tensor_copy(stg[:, 512:1024], tile_full[:, 512:1024])
            nc.sync.dma_start(out_d.ap()[st * 128 : (st + 1) * 128, :], stg[:])

        tpsA = ps.tile([128, 2 * IC], f32, tag="S", name="tpsA")  # st10
        tpsB = ps.tile([128, 2 * IC], f32, tag="S", name="tpsB")  # st11
        for st, t in ((10, tpsA), (11, tpsB)):
            for oc in range(2):
                for kt2 in range(2):
                    omm(t[:, oc * 512 : (oc + 1) * 512], st, kt2, oc)
        t12a = op_.tile([128, 512], f32, tag="O", name="t12a")  # early bank
        t12b = fp.tile([128, 512], f32, tag="f", name="t12b")
        omm(t12a[:], 12, 0, 0)
        omm(t12b[:], 12, 0, 1)
        evict_dma(10, tpsA, "scalar")
        evict_dma(11, tpsB, "vector")
        tpsC = ps.tile([128, 2 * IC], f32, tag="S", name="tpsC")  # st14
        tpsD = ps.tile([128, 2 * IC], f32, tag="S", name="tpsD")  # st15
        for oc in range(2):
            omm(tpsC[:, oc * 512 : (oc + 1) * 512], 14, 0, oc)
            omm(tpsD[:, oc * 512 : (oc + 1) * 512], 15, 0, oc)
        t13a = op_.tile([128, 512], f32, tag="O", name="t13a")  # O-acc banks
        t13b = op_.tile([128, 512], f32, tag="O", name="t13b")
        omm(t13a[:], 13, 0, 0)
        omm(t13b[:], 13, 0, 1)
        omm(t12a[:], 12, 1, 0)
        omm(t12b[:], 12, 1, 1)
        omm(t13a[:], 13, 1, 0)
        omm(t13b[:], 13, 1, 1)
        for oc in range(2):
            omm(tpsC[:, oc * 512 : (oc + 1) * 512], 14, 1, oc)
            omm(tpsD[:, oc * 512 : (oc + 1) * 512], 15, 1, oc)
        stg12 = tlo.tile([128, D], f32, tag="to")
        nc.vector.tensor_copy(stg12[:, 0:512], t12a[:])
        nc.vector.tensor_copy(stg12[:, 512:1024], t12b[:])
        nc.sync.dma_start(out_d.ap()[12 * 128 : 13 * 128, :], stg12[:])
        stg13 = tlo.tile([128, D], f32, tag="to")
        nc.vector.tensor_copy(stg13[:, 0:512], t13a[:])
        nc.vector.tensor_copy(stg13[:, 512:1024], t13b[:])
        nc.sync.dma_start(out_d.ap()[13 * 128 : 14 * 128, :], stg13[:])
        evict_dma(14, tpsC, "scalar")
        stg15 = tlo.tile([128, D], f32, tag="to")
        nc.scalar.activation(stg15[:, 0:512], tpsD[:, 0:512], Act.Copy)
        nc.vector.tensor_copy(stg15[:, 512:1024], tpsD[:, 512:1024])
        nc.sync.dma_start(out_d.ap()[15 * 128 : 16 * 128, :], stg15[:])

    nc.compile()
    return nc


def _get_nc():
    global _nc_cache
    if _nc_cache is None:
        _nc_cache = _build_nc()
    return _nc_cache


def _prepare_in_maps(x, W_q, b_q, W_k, b_k, W_v, b_v, W_o, b_o):
    in_maps = []
    for c in range(N_CORES):
        b, g = c // 4, c % 4
        rows = slice(DL * g, DL * g + DL)
        bqk = np.stack(
            [
                b_q[DL * g : DL * g + 128],
                b_q[DL * g + 128 : DL * g + 256],
                b_k[DL * g : DL * g + 128],
                b_k[DL * g + 128 : DL * g + 256],
            ],
            axis=1,
        ).astype(np.float32)
        # x staged as [p, c, kt, 512]: xt[p, ci, kt, s] = x[b].T[kt*128+p,
        # ci*512+s] so each 512-col chunk is one strided DMA.
        xT = np.ascontiguousarray(x[b].T).astype(_BF16)  # [D, S]
        xt = np.ascontiguousarray(
            xT.reshape(KT, 128, NIC, 512).transpose(1, 2, 0, 3)
        )
        def tile_qk(w):  # [k, dl] -> [p, mhalf, kt, 128]
            return np.ascontiguousarray(
                w.reshape(KT, 128, 2, 128).transpose(1, 2, 0, 3)
            )

        wqT = W_q[rows].T.astype(_BF16)
        wkT = W_k[rows].T.astype(_BF16)
        wvT = W_v[rows].T.astype(_BF16)  # [k, dl]
        woT = W_o[:, rows].T.astype(_BF16)  # [dl, o]
        in_maps.append(
            {
                "xt": xt,
                "wq": tile_qk(wqT),
                "wk": tile_qk(wkT),
                "wv": np.ascontiguousarray(
                    wvT.reshape(KT, 128, DL).transpose(1, 0, 2)
                ),
                "wo": np.ascontiguousarray(
                    woT.reshape(2, 128, D).transpose(1, 0, 2)
                ),
                "bqk": np.ascontiguousarray(bqk),
                "bv": np.ascontiguousarray(
                    np.broadcast_to(b_v[rows], (128, DL))
                ).astype(np.float32),
            }
        )
    return in_maps


def _assemble(results, b_o):
    out = np.empty((B, S, D), dtype=np.float32)
    for b in range(B):
        acc = results[4 * b]["out"].astype(np.float32).copy()
        for g in range(1, 4):
            acc += results[4 * b + g]["out"]
        out[b] = acc + b_o[None, :].astype(np.float32)
    return out


def kernel(x, W_q, b_q, W_k, b_k, W_v, b_v, W_o, b_o):
    from concourse.bass_utils import run_bass_kernel_spmd

    x = np.asarray(x, dtype=np.float32)
    nc = _get_nc()
    in_maps = _prepare_in_maps(
        x,
        np.asarray(W_q, np.float32),
        np.asarray(b_q, np.float32),
        np.asarray(W_k, np.float32),
        np.asarray(b_k, np.float32),
        np.asarray(W_v, np.float32),
        np.asarray(b_v, np.float32),
        np.asarray(W_o, np.float32),
        np.asarray(b_o, np.float32),
    )
    res = run_bass_kernel_spmd(nc, in_maps, core_ids=list(range(N_CORES)))
    return _assemble(res.results, np.asarray(b_o, np.float32))


# revision 22
# speedup vs baseline: 1.0010x; 1.0010x over previous
"""Multi-head attention (B=2, S=2048, D=1024, H=16) on 8 TRN2 NeuronCores.

Sharding: data parallel on batch (2) x tensor parallel on heads (4 groups of
4 heads).  Core c handles batch c//4, heads 4*(c%4) .. 4*(c%4)+4.  Each core
computes q/k/v projections for its 256 output dims, attention for its 4
heads, and a partial (row-parallel) output projection.  The host sums the 4
partials per batch and adds b_o.

Per-core kernel (all matmuls bf16 inputs, fp32 PSUM):
  - qT/kT d-major [256, 2048]; v s-major with a ones column per head
    zero-padded to 128 columns (full-array PV keeps the PE HAM clock
    un-throttled).  Even heads keep v at dd 0:64 with ones at 64 (PV out
    rows 0:64 data, 64 den); odd heads put v at dd 64:128 with ones at 63
    (PV out rows 64:128 data, 63 den) so the softmax normalization can
    write aoT in place on the Vector engine - no partition-shift DMA
    (den rows 64/32: partition bases must be 32-aligned).
  - scores are computed transposed (S[j, i] = k_j . q_i): no transposes
    anywhere.  The two heads of a pair run as K=64 matmuls on distinct PE
    row-groups (base partitions 0/64) writing the two halves of one shared
    PSUM tile - they execute concurrently, so the array stays fully busy and
    the score cost halves vs zero-padding.
  - softmax exp runs on the Scalar engine straight out of PSUM (one
    activation covers both heads); no max-subtraction (scores std ~0.33).
  - i-chunk 512: S-pair tile [128,1024] double-buffered (4 banks) + three
    [128,512] O accumulators (3) + one filler bank = exactly 8 PSUM banks.
  - projections / output-projection groups are emitted as fillers inside the
    attention steps so the PE works while the Scalar engine streams exps.
  - x is staged host-side as [p, c, kt*512] so each 512-col chunk loads with
    one (c>0) or eight (c=0, per-kt for progressive consumption) DMAs; w_k
    and w_q load in m-halves so the first matmul's weights land early.
  - a short burst of dummy matmuls on zeroed SBUF runs during the DMA
    lead-in so the PE activity manager ramps the clock before real work.
  - the last four output tiles (st 12-15) run phase-major after the last
    attention chunk: all kt2=0 partial matmuls issue first (they overlap the
    final softmax-normalization chain), then kt2=1, into 8 PSUM banks;
    evictions split across Scalar and Vector; one output DMA per tile.
"""

import os

import numpy as np
import ml_dtypes

B, S, D = 2, 2048, 1024
H, DH = 16, 64
N_CORES = 8
HPC = 4  # heads per core
DL = HPC * DH  # 256 local dims per core
KT = D // 128  # 8 k-tiles
ST = S // 128  # 16 s-tiles (also j-tiles)
IC = 512  # i-chunk (query chunk)
NIC = S // IC

_BF16 = ml_dtypes.bfloat16

_nc_cache = None


def _build_nc():
    from contextlib import ExitStack

    import concourse.mybir as mybir
    import concourse.tile as tile
    from concourse import bacc

    f32 = mybir.dt.float32
    bf16 = mybir.dt.bfloat16
    Alu = mybir.AluOpType
    Act = mybir.ActivationFunctionType

    nc = bacc.Bacc("TRN2", target_bir_lowering=False, debug=False, enable_asserts=False)

    xt_d = nc.dram_tensor("xt", (128, NIC, KT, 512), bf16, kind="ExternalInput")
    wq_d = nc.dram_tensor("wq", (128, 2, KT, 128), bf16, kind="ExternalInput")
    wk_d = nc.dram_tensor("wk", (128, 2, KT, 128), bf16, kind="ExternalInput")
    wv_d = nc.dram_tensor("wv", (128, KT, DL), bf16, kind="ExternalInput")
    wo_d = nc.dram_tensor("wo", (128, 2, D), bf16, kind="ExternalInput")
    bqk_d = nc.dram_tensor("bqk", (128, 4), f32, kind="ExternalInput")
    bv_d = nc.dram_tensor("bv", (128, DL), f32, kind="ExternalInput")
    out_d = nc.dram_tensor("out", (S, D), f32, kind="ExternalOutput")

    with tile.TileContext(nc) as tc, ExitStack() as ctx:
        consts = ctx.enter_context(tc.tile_pool(name="consts", bufs=1))
        xbf = consts.tile([128, NIC, KT, 512], bf16)  # [p, c, kt, s-in-chunk]
        wq_sb = consts.tile([128, 2, KT, 128], bf16)  # [p, mhalf, kt, m]
        wk_sb = consts.tile([128, 2, KT, 128], bf16)
        wv_sb = consts.tile([128, KT, DL], bf16)
        wo_sb = consts.tile([128, 2, D], bf16)  # [p, kt2, o]
        bqk_sb = consts.tile([128, 4], f32)
        bv_sb = consts.tile([128, DL], f32)
        qT = consts.tile([128, 2, S], bf16)  # [p, mt, s]
        kT = consts.tile([128, 2, S], bf16)
        # v (s-major), ones column per head, zero-padded to 128 cols.
        # even h: v at 0:64, ones at 64; odd h: ones at 0, v at 64:128.
        vaug = consts.tile([128, ST, HPC, 128], bf16)  # [p(j), jt, h, dd]
        aoT = consts.tile([128, 2, S], bf16)  # attn-out transposed [p, kt2, s]
        dum = consts.tile([128, 512], bf16)  # warm-up matmul operand

        # ---- input DMAs: everything is host-pre-tiled to [p, ...] layouts
        # so each load is one DMA with fat (multi-KB) contiguous descriptors.
        # wk m-half 0 + x chunk 0 gate the first matmul.
        nc.sync.dma_start(wk_sb[:, 0], wk_d.ap()[:, 0])
        nc.sync.dma_start(xbf[:, 0], xt_d.ap()[:, 0])
        nc.sync.dma_start(wq_sb[:, 0], wq_d.ap()[:, 0])
        nc.sync.dma_start(bqk_sb[:], bqk_d.ap())
        nc.sync.dma_start(wv_sb[:], wv_d.ap())
        nc.sync.dma_start(bv_sb[:], bv_d.ap())
        nc.sync.dma_start(xbf[:, 1], xt_d.ap()[:, 1])
        nc.sync.dma_start(wk_sb[:, 1], wk_d.ap()[:, 1])
        nc.sync.dma_start(wq_sb[:, 1], wq_d.ap()[:, 1])
        nc.sync.dma_start(xbf[:, 2], xt_d.ap()[:, 2])
        nc.sync.dma_start(xbf[:, 3], xt_d.ap()[:, 3])
        nc.sync.dma_start(wo_sb[:], wo_d.ap())

        nc.gpsimd.memset(dum[:], 0.0)
        for h in range(HPC):
            if h % 2 == 0:
                nc.gpsimd.memset(vaug[:, :, h, DH + 1 :], 0.0)
                nc.gpsimd.memset(vaug[:, :, h, DH : DH + 1], 1.0)
            else:
                nc.gpsimd.memset(vaug[:, :, h, 0:1], 1.0)
                nc.gpsimd.memset(vaug[:, :, h, 1:DH], 0.0)

        # Preload the exp activation table set (~2.7us) during the DMA
        # lead-in so the first real softmax exp doesn't pay for it.
        warm = consts.tile([128, 8], f32)
        nc.gpsimd.memset(warm[:], 0.0)
        nc.scalar.activation(warm[:], warm[:], Act.Exp)

        ps = ctx.enter_context(tc.tile_pool(name="ps", bufs=2, space="PSUM"))
        op_ = ctx.enter_context(tc.tile_pool(name="op", bufs=3, space="PSUM"))
        fp = ctx.enter_context(tc.tile_pool(name="fp", bufs=1, space="PSUM"))
        ep = ctx.enter_context(tc.tile_pool(name="ep", bufs=8))
        rp = ctx.enter_context(tc.tile_pool(name="rp", bufs=3))
        tp = ctx.enter_context(tc.tile_pool(name="tp", bufs=3))
        osb = ctx.enter_context(tc.tile_pool(name="osb", bufs=3))
        tlo = ctx.enter_context(tc.tile_pool(name="tlo", bufs=4))

        def qk_proj(proj, mt, c, alt=False, kts=None):
            """q (proj=0) / k (proj=1) projection, one 512-col chunk."""
            w_sb = wq_sb if proj == 0 else wk_sb
            dst_all = qT if proj == 0 else kT
            pool, tg = (op_, "O") if alt else (fp, "f")
            p = pool.tile([128, 512], f32, tag=tg)
            for kt in kts if kts is not None else range(KT):
                nc.tensor.matmul(
                    p[:],
                    w_sb[:, mt, kt, :],
                    xbf[:, c, kt, :],
                    start=(kt == 0),
                    stop=(kt == KT - 1),
                )
            dst = dst_all[:, mt, c * 512 : (c + 1) * 512]
            bias_ap = bqk_sb[:, proj * 2 + mt : proj * 2 + mt + 1]
            if proj == 0:
                nc.vector.tensor_scalar(dst, p[:], bias_ap, 0.125, Alu.add, Alu.mult)
            else:
                nc.vector.tensor_scalar(dst, p[:], bias_ap, None, Alu.add)

        def v_proj(st):
            pool = fp if st % 2 == 0 else op_
            p = pool.tile([128, DL], f32, tag="f" if st % 2 == 0 else "O")
            for kt in range(KT):
                nc.tensor.matmul(
                    p[:],
                    xbf[:, st // 4, kt, (st % 4) * 128 : (st % 4 + 1) * 128],
                    wv_sb[:, kt, :],
                    start=(kt == 0),
                    stop=(kt == KT - 1),
                )
            for h in range(HPC):
                dd = 0 if h % 2 == 0 else DH
                nc.vector.tensor_tensor(
                    vaug[:, st, h, dd : dd + DH],
                    p[:, h * DH : (h + 1) * DH],
                    bv_sb[:, h * DH : (h + 1) * DH],
                    Alu.add,
                )

        def o_proj_chunk(st, oc):
            pso = fp.tile([128, 512], f32, tag="f")
            for kt2 in range(2):
                nc.tensor.matmul(
                    pso[:],
                    aoT[:, kt2, st * 128 : (st + 1) * 128],
                    wo_sb[:, kt2, oc * 512 : (oc + 1) * 512],
                    start=(kt2 == 0),
                    stop=(kt2 == 1),
                )
            stg = osb.tile([128, 512], f32, tag="oh")
            nc.vector.tensor_copy(stg[:], pso[:])
            nc.sync.dma_start(
                out_d.ap()[st * 128 : (st + 1) * 128, oc * 512 : (oc + 1) * 512],
                stg[:],
            )

        def attn_norm(h, ic, O, direct=False):
            # even h: data rows 0:64, den row 64; odd h: data rows 64:128,
            # den row 0.  direct=True: the normalize multiply writes aoT in
            # place (same partition base) - used for the last chunk so the
            # tail o-proj starts sooner.  Otherwise stage via tmp + DMA (the
            # scheduler pipelines that better against o-proj fillers).
            pb, mt = 64 * (h % 2), h // 2
            den_row = DH if h % 2 == 0 else 0
            den = rp.tile([1, IC], f32, tag="den")
            if direct and h % 2 == 1:
                # Scalar engine copies head B's denominator so it doesn't
                # queue behind head A's chain on the Vector engine.
                nc.scalar.activation(den[:], O[den_row : den_row + 1, :], Act.Copy)
            else:
                nc.vector.tensor_copy(den[:], O[den_row : den_row + 1, :])
            recip = rp.tile([1, IC], f32, tag="r")
            nc.vector.reciprocal_approx_fast(recip[:], den[:])
            rb = rp.tile([64, IC], f32, tag="rb")
            nc.gpsimd.partition_broadcast(rb[:], recip[:])
            if direct:
                nc.vector.tensor_tensor(
                    aoT[pb : pb + 64, mt, ic * IC : (ic + 1) * IC],
                    O[pb : pb + 64, :],
                    rb[:],
                    Alu.mult,
                )
            else:
                tmp = tp.tile([64, IC], bf16, tag="t")
                nc.vector.tensor_tensor(tmp[:], O[pb : pb + 64, :], rb[:], Alu.mult)
                nc.sync.dma_start(
                    aoT[pb : pb + 64, mt, ic * IC : (ic + 1) * IC], tmp[:]
                )

        def pair_ic(pair, ic, fillers, mid_fillers={}, direct_norm=False):
            """Attention for head pair (2*pair, 2*pair+1) on query chunk ic.
            fillers: {jt: [callable, ...]} emitted just before that step;
            mid_fillers: emitted between the step's exp and its PV.
            The PV matmuls run one jt behind the scores/exp so the PE
            streams scores(jt+1) while exp(jt) is still on the Scalar
            engine instead of blocking in-order on PV(jt)."""
            hA, hB = 2 * pair, 2 * pair + 1
            OA = op_.tile([128, IC], f32, tag="O")
            OB = op_.tile([128, IC], f32, tag="O")
            prevE = None

            def pv(jt, E):
                nc.tensor.matmul(
                    OA[:],
                    vaug[:, jt, hA, :],
                    E[:, 0:IC],
                    start=(jt == 0),
                    stop=(jt == ST - 1),
                )
                nc.tensor.matmul(
                    OB[:],
                    vaug[:, jt, hB, :],
                    E[:, IC : 2 * IC],
                    start=(jt == 0),
                    stop=(jt == ST - 1),
                )

            for jt in range(ST):
                for f in fillers.get(jt, ()):
                    f()
                Sp = ps.tile([128, 2 * IC], f32, tag="S")
                nc.tensor.matmul(
                    Sp[:, 0:IC],
                    kT[0:64, pair, jt * 128 : (jt + 1) * 128],
                    qT[0:64, pair, ic * IC : (ic + 1) * IC],
                    start=True,
                    stop=True,
                )
                nc.tensor.matmul(
                    Sp[:, IC : 2 * IC],
                    kT[64:128, pair, jt * 128 : (jt + 1) * 128],
                    qT[64:128, pair, ic * IC : (ic + 1) * IC],
                    start=True,
                    stop=True,
                )
                E = ep.tile([128, 2 * IC], bf16, tag="E")
                nc.scalar.activation(E[:], Sp[:], Act.Exp)
                for f in mid_fillers.get(jt, ()):
                    f()
                if prevE is not None:
                    pv(jt - 1, prevE)
                prevE = E
            pv(ST - 1, prevE)
            attn_norm(hA, ic, OA, direct=direct_norm)
            attn_norm(hB, ic, OB, direct=direct_norm)

        # ---- emission schedule ----
        qk_proj(1, 0, 0)
        qk_proj(0, 0, 0, alt=True)
        v_proj(0)
        v_proj(1)
        F = lambda *fs: list(fs)
        p0i0 = {jt: F(lambda st=jt + 2: v_proj(st)) for jt in range(ST - 2)}
        for jt, c in ((2, 1), (5, 2), (9, 3)):
            p0i0[jt] = [lambda c=c: qk_proj(1, 0, c)] + p0i0[jt]
        p0i0[12] = [lambda: qk_proj(0, 0, 1)] + p0i0[12]
        pair_ic(0, 0, p0i0)
        pair_ic(0, 1, {
            4: F(lambda: qk_proj(1, 1, 0)),
            6: F(lambda: qk_proj(1, 1, 1)),
            8: F(lambda: qk_proj(1, 1, 2)),
            10: F(lambda: qk_proj(1, 1, 3)),
            13: F(lambda: qk_proj(0, 1, 0)),
        })
        pair_ic(1, 0, {
            4: F(lambda: qk_proj(0, 1, 1)),
            9: F(lambda: qk_proj(0, 0, 2)),
        })
        pair_ic(1, 1, {
            4: F(lambda: qk_proj(0, 1, 2)),
            9: F(lambda: qk_proj(0, 0, 3)),
            12: F(lambda: qk_proj(0, 1, 3)),
        })
        pair_ic(0, 2, {5 + i: F(lambda st=(i + 2) // 2, oc=i % 2: o_proj_chunk(st, oc))
                       for i in range(6)})
        pair_ic(1, 2, {4 + i: F(lambda st=4 + i // 2, oc=i % 2: o_proj_chunk(st, oc))
                       for i in range(8)})
        pair_ic(0, 3, {4 + i: F(lambda st=8 + i // 2, oc=i % 2: o_proj_chunk(st, oc))
                       for i in range(4)})
        pair_ic(1, 3, {4: F(lambda: o_proj_chunk(0, 0)),
                       8: F(lambda: o_proj_chunk(0, 1))}, direct_norm=True)

        # ---- dense tail: st 10-15 across all 8 PSUM banks, ordered by when
        # each bank frees.  st10/11 (whose aoT inputs are long ready) and the
        # st12 kt2=0 partials run while the final softmax-normalization chain
        # is on Vector/GpSimd; the ps banks are then re-used for st14/15.
        # st13 sits in the two banks the last pair's O accumulators vacate at
        # the norm multiplies.  kt2=1 for st12-15 follows the norm.
        def omm(piece, st, kt2, oc):
            nc.tensor.matmul(
                piece,
                aoT[:, kt2, st * 128 : (st + 1) * 128],
                wo_sb[:, kt2, oc * 512 : (oc + 1) * 512],
                start=(kt2 == 0),
                stop=(kt2 == 1),
            )

        def evict_dma(st, tile_full, engine):
            stg = tlo.tile([128, D], f32, tag="to")
            if engine == "scalar":
                nc.scalar.activation(stg[:], tile_full[:], Act.Copy)
            else:
                nc.vector.tensor_copy(stg[:, 0:512], tile_full[:, 0:512])
                nc.vector.tensor_copy(stg[:, 512:1024], tile_full[:, 512:1024])
            nc.sync.dma_start(out_d.ap()[st * 128 : (st + 1) * 128, :], stg[:])

        tpsA = ps.tile([128, 2 * IC], f32, tag="S", name="tpsA")  # st10
        tpsB = ps.tile([128, 2 * IC], f32, tag="S", name="tpsB")  # st11
        for st, t in ((10, tpsA), (11, tpsB)):
            for oc in range(2):
                for kt2 in range(2):
                    omm(t[:, oc * 512 : (oc + 1) * 512], st, kt2, oc)
        t12a = op_.tile([128, 512], f32, tag="O", name="t12a")  # early bank
        t12b = fp.tile([128, 512], f32, tag="f", name="t12b")
        omm(t12a[:], 12, 0, 0)
        omm(t12b[:], 12, 0, 1)
        evict_dma(10, tpsA, "scalar")
        evict_dma(11, tpsB, "vector")
        tpsC = ps.tile([128, 2 * IC], f32, tag="S", name="tpsC")  # st14
        tpsD = ps.tile([128, 2 * IC], f32, tag="S", name="tpsD")  # st15
        for oc in range(2):
            omm(tpsC[:, oc * 512 : (oc + 1) * 512], 14, 0, oc)
            omm(tpsD[:, oc * 512 : (oc + 1) * 512], 15, 0, oc)
        t13a = op_.tile([128, 512], f32, tag="O", name="t13a")  # O-acc banks
        t13b = op_.tile([128, 512], f32, tag="O", name="t13b")
        omm(t13a[:], 13, 0, 0)
        omm(t13b[:], 13, 0, 1)
        omm(t12a[:], 12, 1, 0)
        omm(t12b[:], 12, 1, 1)
        omm(t13a[:], 13, 1, 0)
        omm(t13b[:], 13, 1, 1)
        for oc in range(2):
            omm(tpsC[:, oc * 512 : (oc + 1) * 512], 14, 1, oc)
            omm(tpsD[:, oc * 512 : (oc + 1) * 512], 15, 1, oc)
        stg12 = tlo.tile([128, D], f32, tag="to")
        nc.vector.tensor_copy(stg12[:, 0:512], t12a[:])
        nc.vector.tensor_copy(stg12[:, 512:1024], t12b[:])
        nc.sync.dma_start(out_d.ap()[12 * 128 : 13 * 128, :], stg12[:])
        stg13 = tlo.tile([128, D], f32, tag="to")
        nc.vector.tensor_copy(stg13[:, 0:512], t13a[:])
        nc.vector.tensor_copy(stg13[:, 512:1024], t13b[:])
        nc.sync.dma_start(out_d.ap()[13 * 128 : 14 * 128, :], stg13[:])
        evict_dma(14, tpsC, "scalar")
        evict_dma(15, tpsD, "scalar")

    nc.compile()
    return nc


def _get_nc():
    global _nc_cache
    if _nc_cache is None:
        _nc_cache = _build_nc()
    return _nc_cache


def _prepare_in_maps(x, W_q, b_q, W_k, b_k, W_v, b_v, W_o, b_o):
    in_maps = []
    for c in range(N_CORES):
        b, g = c // 4, c % 4
        rows = slice(DL * g, DL * g + DL)
        bqk = np.stack(
            [
                b_q[DL * g : DL * g + 128],
                b_q[DL * g + 128 : DL * g + 256],
                b_k[DL * g : DL * g + 128],
                b_k[DL * g + 128 : DL * g + 256],
            ],
            axis=1,
        ).astype(np.float32)
        # x staged as [p, c, kt, 512]: xt[p, ci, kt, s] = x[b].T[kt*128+p,
        # ci*512+s] so each 512-col chunk is one strided DMA.
        xT = np.ascontiguousarray(x[b].T).astype(_BF16)  # [D, S]
        xt = np.ascontiguousarray(
            xT.reshape(KT, 128, NIC, 512).transpose(1, 2, 0, 3)
        )
        def tile_qk(w):  # [k, dl] -> [p, mhalf, kt, 128]
            return np.ascontiguousarray(
                w.reshape(KT, 128, 2, 128).transpose(1, 2, 0, 3)
            )

        wqT = W_q[rows].T.astype(_BF16)
        wkT = W_k[rows].T.astype(_BF16)
        wvT = W_v[rows].T.astype(_BF16)  # [k, dl]
        woT = W_o[:, rows].T.astype(_BF16)  # [dl, o]
        in_maps.append(
            {
                "xt": xt,
                "wq": tile_qk(wqT),
                "wk": tile_qk(wkT),
                "wv": np.ascontiguousarray(
                    wvT.reshape(KT, 128, DL).transpose(1, 0, 2)
                ),
                "wo": np.ascontiguousarray(
                    woT.reshape(2, 128, D).transpose(1, 0, 2)
                ),
                "bqk": np.ascontiguousarray(bqk),
                "bv": np.ascontiguousarray(
                    np.broadcast_to(b_v[rows], (128, DL))
                ).astype(np.float32),
            }
        )
    return in_maps


def _assemble(results, b_o):
    out = np.empty((B, S, D), dtype=np.float32)
    for b in range(B):
        acc = results[4 * b]["out"].astype(np.float32).copy()
        for g in range(1, 4):
            acc += results[4 * b + g]["out"]
        out[b] = acc + b_o[None, :].astype(np.float32)
    return out


def kernel(x, W_q, b_q, W_k, b_k, W_v, b_v, W_o, b_o):
    from concourse.bass_utils import run_bass_kernel_spmd

    x = np.asarray(x, dtype=np.float32)
    nc = _get_nc()
    in_maps = _prepare_in_maps(
        x,
        np.asarray(W_q, np.float32),
        np.asarray(b_q, np.float32),
        np.asarray(W_k, np.float32),
        np.asarray(b_k, np.float32),
        np.asarray(W_v, np.float32),
        np.asarray(b_v, np.float32),
        np.asarray(W_o, np.float32),
        np.asarray(b_o, np.float32),
    )
    res = run_bass_kernel_spmd(nc, in_maps, core_ids=list(range(N_CORES)))
    return _assemble(res.results, np.asarray(b_o, np.float32))


# revision 23
# speedup vs baseline: 1.0123x; 1.0113x over previous
"""Multi-head attention (B=2, S=2048, D=1024, H=16) on 8 TRN2 NeuronCores.

Sharding: data parallel on batch (2) x tensor parallel on heads (4 groups of
4 heads).  Core c handles batch c//4, heads 4*(c%4) .. 4*(c%4)+4.  Each core
computes q/k/v projections for its 256 output dims, attention for its 4
heads, and a partial (row-parallel) output projection.  The host sums the 4
partials per batch and adds b_o.

Per-core kernel (all matmuls bf16 inputs, fp32 PSUM):
  - qT/kT d-major [256, 2048]; v s-major with a ones column per head
    zero-padded to 128 columns (full-array PV keeps the PE HAM clock
    un-throttled).  Even heads keep v at dd 0:64 with ones at 64 (PV out
    rows 0:64 data, 64 den); odd heads put v at dd 64:128 with ones at 63
    (PV out rows 64:128 data, 63 den) so the softmax normalization can
    write aoT in place on the Vector engine - no partition-shift DMA
    (den rows 64/32: partition bases must be 32-aligned).
  - scores are computed transposed (S[j, i] = k_j . q_i): no transposes
    anywhere.  The two heads of a pair run as K=64 matmuls on distinct PE
    row-groups (base partitions 0/64) writing the two halves of one shared
    PSUM tile - they execute concurrently, so the array stays fully busy and
    the score cost halves vs zero-padding.
  - softmax exp runs on the Scalar engine straight out of PSUM (one
    activation covers both heads); no max-subtraction (scores std ~0.33).
  - i-chunk 512: S-pair tile [128,1024] double-buffered (4 banks) + three
    [128,512] O accumulators (3) + one filler bank = exactly 8 PSUM banks.
  - projections / output-projection groups are emitted as fillers inside the
    attention steps so the PE works while the Scalar engine streams exps.
  - x is staged host-side as [p, c, kt*512] so each 512-col chunk loads with
    one (c>0) or eight (c=0, per-kt for progressive consumption) DMAs; w_k
    and w_q load in m-halves so the first matmul's weights land early.
  - a short burst of dummy matmuls on zeroed SBUF runs during the DMA
    lead-in so the PE activity manager ramps the clock before real work.
  - the last four output tiles (st 12-15) run phase-major after the last
    attention chunk: all kt2=0 partial matmuls issue first (they overlap the
    final softmax-normalization chain), then kt2=1, into 8 PSUM banks;
    evictions split across Scalar and Vector; one output DMA per tile.
"""

import os

import numpy as np
import ml_dtypes

B, S, D = 2, 2048, 1024
H, DH = 16, 64
N_CORES = 8
HPC = 4  # heads per core
DL = HPC * DH  # 256 local dims per core
KT = D // 128  # 8 k-tiles
ST = S // 128  # 16 s-tiles (also j-tiles)
IC = 512  # i-chunk (query chunk)
NIC = S // IC

_BF16 = ml_dtypes.bfloat16

_nc_cache = None


def _build_nc():
    from contextlib import ExitStack

    import concourse.mybir as mybir
    import concourse.tile as tile
    from concourse import bacc

    f32 = mybir.dt.float32
    bf16 = mybir.dt.bfloat16
    Alu = mybir.AluOpType
    Act = mybir.ActivationFunctionType

    nc = bacc.Bacc("TRN2", target_bir_lowering=False, debug=False, enable_asserts=False)

    xt_d = nc.dram_tensor("xt", (128, NIC, KT, 512), bf16, kind="ExternalInput")
    wq_d = nc.dram_tensor("wq", (128, 2, KT, 128), bf16, kind="ExternalInput")
    wk_d = nc.dram_tensor("wk", (128, 2, KT, 128), bf16, kind="ExternalInput")
    wv_d = nc.dram_tensor("wv", (128, KT, DL), bf16, kind="ExternalInput")
    wo_d = nc.dram_tensor("wo", (128, 2, D), bf16, kind="ExternalInput")
    bqk_d = nc.dram_tensor("bqk", (128, 4), f32, kind="ExternalInput")
    bv_d = nc.dram_tensor("bv", (128, DL), f32, kind="ExternalInput")
    out_d = nc.dram_tensor("out", (S, D), f32, kind="ExternalOutput")

    with tile.TileContext(nc) as tc, ExitStack() as ctx:
        consts = ctx.enter_context(tc.tile_pool(name="consts", bufs=1))
        xbf = consts.tile([128, NIC, KT, 512], bf16)  # [p, c, kt, s-in-chunk]
        wq_sb = consts.tile([128, 2, KT, 128], bf16)  # [p, mhalf, kt, m]
        wk_sb = consts.tile([128, 2, KT, 128], bf16)
        wv_sb = consts.tile([128, KT, DL], bf16)
        wo_sb = consts.tile([128, 2, D], bf16)  # [p, kt2, o]
        bqk_sb = consts.tile([128, 4], f32)
        bv_sb = consts.tile([128, DL], f32)
        qT = consts.tile([128, 2, S], bf16)  # [p, mt, s]
        kT = consts.tile([128, 2, S], bf16)
        # v (s-major), ones column per head, zero-padded to 128 cols.
        # even h: v at 0:64, ones at 64; odd h: ones at 0, v at 64:128.
        vaug = consts.tile([128, ST, HPC, 128], bf16)  # [p(j), jt, h, dd]
        aoT = consts.tile([128, 2, S], bf16)  # attn-out transposed [p, kt2, s]
        dum = consts.tile([128, 512], bf16)  # warm-up matmul operand

        # ---- input DMAs: everything is host-pre-tiled to [p, ...] layouts
        # so each load is one DMA with fat (multi-KB) contiguous descriptors.
        # wk m-half 0 + x chunk 0 gate the first matmul.
        nc.sync.dma_start(wk_sb[:, 0], wk_d.ap()[:, 0])
        nc.sync.dma_start(xbf[:, 0], xt_d.ap()[:, 0])
        nc.sync.dma_start(wq_sb[:, 0], wq_d.ap()[:, 0])
        nc.sync.dma_start(bqk_sb[:], bqk_d.ap())
        nc.sync.dma_start(wv_sb[:], wv_d.ap())
        nc.sync.dma_start(bv_sb[:], bv_d.ap())
        nc.sync.dma_start(xbf[:, 1], xt_d.ap()[:, 1])
        nc.sync.dma_start(wk_sb[:, 1], wk_d.ap()[:, 1])
        nc.sync.dma_start(wq_sb[:, 1], wq_d.ap()[:, 1])
        nc.sync.dma_start(xbf[:, 2], xt_d.ap()[:, 2])
        nc.sync.dma_start(xbf[:, 3], xt_d.ap()[:, 3])
        nc.sync.dma_start(wo_sb[:], wo_d.ap())

        nc.gpsimd.memset(dum[:], 0.0)
        for h in range(HPC):
            if h % 2 == 0:
                nc.gpsimd.memset(vaug[:, :, h, DH + 1 :], 0.0)
                nc.gpsimd.memset(vaug[:, :, h, DH : DH + 1], 1.0)
            else:
                nc.gpsimd.memset(vaug[:, :, h, 0:1], 1.0)
                nc.gpsimd.memset(vaug[:, :, h, 1:DH], 0.0)

        # Preload the exp activation table set (~2.7us) during the DMA
        # lead-in so the first real softmax exp doesn't pay for it.
        warm = consts.tile([128, 8], f32)
        nc.gpsimd.memset(warm[:], 0.0)
        nc.scalar.activation(warm[:], warm[:], Act.Exp)

        ps = ctx.enter_context(tc.tile_pool(name="ps", bufs=2, space="PSUM"))
        op_ = ctx.enter_context(tc.tile_pool(name="op", bufs=3, space="PSUM"))
        fp = ctx.enter_context(tc.tile_pool(name="fp", bufs=1, space="PSUM"))
        ep = ctx.enter_context(tc.tile_pool(name="ep", bufs=8))
        rp = ctx.enter_context(tc.tile_pool(name="rp", bufs=3))
        tp = ctx.enter_context(tc.tile_pool(name="tp", bufs=3))
        osb = ctx.enter_context(tc.tile_pool(name="osb", bufs=3))
        tlo = ctx.enter_context(tc.tile_pool(name="tlo", bufs=4))

        # Clock warm-up: tiny matmuls in the fp bank during the DMA lead-in
        # so the PE activity manager ramps before real work arrives; they
        # complete before the first weights land, so they delay nothing.
        wm = fp.tile([128, 512], f32, tag="f", name="wm")
        for _ in range(24):
            nc.tensor.matmul(
                wm[:, 0:64], dum[:, 0:128], dum[:, 0:64], start=True, stop=True
            )

        def qk_proj(proj, mt, c, alt=False, kts=None):
            """q (proj=0) / k (proj=1) projection, one 512-col chunk."""
            w_sb = wq_sb if proj == 0 else wk_sb
            dst_all = qT if proj == 0 else kT
            pool, tg = (op_, "O") if alt else (fp, "f")
            p = pool.tile([128, 512], f32, tag=tg)
            for kt in kts if kts is not None else range(KT):
                nc.tensor.matmul(
                    p[:],
                    w_sb[:, mt, kt, :],
                    xbf[:, c, kt, :],
                    start=(kt == 0),
                    stop=(kt == KT - 1),
                )
            dst = dst_all[:, mt, c * 512 : (c + 1) * 512]
            bias_ap = bqk_sb[:, proj * 2 + mt : proj * 2 + mt + 1]
            if proj == 0:
                nc.vector.tensor_scalar(dst, p[:], bias_ap, 0.125, Alu.add, Alu.mult)
            else:
                nc.vector.tensor_scalar(dst, p[:], bias_ap, None, Alu.add)

        def v_proj(st):
            pool = fp if st % 2 == 0 else op_
            p = pool.tile([128, DL], f32, tag="f" if st % 2 == 0 else "O")
            for kt in range(KT):
                nc.tensor.matmul(
                    p[:],
                    xbf[:, st // 4, kt, (st % 4) * 128 : (st % 4 + 1) * 128],
                    wv_sb[:, kt, :],
                    start=(kt == 0),
                    stop=(kt == KT - 1),
                )
            for h in range(HPC):
                dd = 0 if h % 2 == 0 else DH
                nc.vector.tensor_tensor(
                    vaug[:, st, h, dd : dd + DH],
                    p[:, h * DH : (h + 1) * DH],
                    bv_sb[:, h * DH : (h + 1) * DH],
                    Alu.add,
                )

        def o_proj_chunk(st, oc):
            pso = fp.tile([128, 512], f32, tag="f")
            for kt2 in range(2):
                nc.tensor.matmul(
                    pso[:],
                    aoT[:, kt2, st * 128 : (st + 1) * 128],
                    wo_sb[:, kt2, oc * 512 : (oc + 1) * 512],
                    start=(kt2 == 0),
                    stop=(kt2 == 1),
                )
            stg = osb.tile([128, 512], f32, tag="oh")
            nc.vector.tensor_copy(stg[:], pso[:])
            nc.sync.dma_start(
                out_d.ap()[st * 128 : (st + 1) * 128, oc * 512 : (oc + 1) * 512],
                stg[:],
            )

        def attn_norm(h, ic, O, direct=False):
            # even h: data rows 0:64, den row 64; odd h: data rows 64:128,
            # den row 0.  direct=True: the normalize multiply writes aoT in
            # place (same partition base) - used for the last chunk so the
            # tail o-proj starts sooner.  Otherwise stage via tmp + DMA (the
            # scheduler pipelines that better against o-proj fillers).
            pb, mt = 64 * (h % 2), h // 2
            den_row = DH if h % 2 == 0 else 0
            den = rp.tile([1, IC], f32, tag="den")
            if direct and h % 2 == 1:
                # Scalar engine copies head B's denominator so it doesn't
                # queue behind head A's chain on the Vector engine.
                nc.scalar.activation(den[:], O[den_row : den_row + 1, :], Act.Copy)
            else:
                nc.vector.tensor_copy(den[:], O[den_row : den_row + 1, :])
            recip = rp.tile([1, IC], f32, tag="r")
            nc.vector.reciprocal_approx_fast(recip[:], den[:])
            rb = rp.tile([64, IC], f32, tag="rb")
            nc.gpsimd.partition_broadcast(rb[:], recip[:])
            if direct:
                nc.vector.tensor_tensor(
                    aoT[pb : pb + 64, mt, ic * IC : (ic + 1) * IC],
                    O[pb : pb + 64, :],
                    rb[:],
                    Alu.mult,
                )
            else:
                tmp = tp.tile([64, IC], bf16, tag="t")
                nc.vector.tensor_tensor(tmp[:], O[pb : pb + 64, :], rb[:], Alu.mult)
                nc.sync.dma_start(
                    aoT[pb : pb + 64, mt, ic * IC : (ic + 1) * IC], tmp[:]
                )

        def pair_ic(pair, ic, fillers, mid_fillers={}, direct_norm=False):
            """Attention for head pair (2*pair, 2*pair+1) on query chunk ic.
            fillers: {jt: [callable, ...]} emitted just before that step;
            mid_fillers: emitted between the step's exp and its PV.
            The PV matmuls run one jt behind the scores/exp so the PE
            streams scores(jt+1) while exp(jt) is still on the Scalar
            engine instead of blocking in-order on PV(jt)."""
            hA, hB = 2 * pair, 2 * pair + 1
            OA = op_.tile([128, IC], f32, tag="O")
            OB = op_.tile([128, IC], f32, tag="O")
            prevE = None

            def pv(jt, E):
                nc.tensor.matmul(
                    OA[:],
                    vaug[:, jt, hA, :],
                    E[:, 0:IC],
                    start=(jt == 0),
                    stop=(jt == ST - 1),
                )
                nc.tensor.matmul(
                    OB[:],
                    vaug[:, jt, hB, :],
                    E[:, IC : 2 * IC],
                    start=(jt == 0),
                    stop=(jt == ST - 1),
                )

            for jt in range(ST):
                for f in fillers.get(jt, ()):
                    f()
                Sp = ps.tile([128, 2 * IC], f32, tag="S")
                nc.tensor.matmul(
                    Sp[:, 0:IC],
                    kT[0:64, pair, jt * 128 : (jt + 1) * 128],
                    qT[0:64, pair, ic * IC : (ic + 1) * IC],
                    start=True,
                    stop=True,
                )
                nc.tensor.matmul(
                    Sp[:, IC : 2 * IC],
                    kT[64:128, pair, jt * 128 : (jt + 1) * 128],
                    qT[64:128, pair, ic * IC : (ic + 1) * IC],
                    start=True,
                    stop=True,
                )
                E = ep.tile([128, 2 * IC], bf16, tag="E")
                nc.scalar.activation(E[:], Sp[:], Act.Exp)
                for f in mid_fillers.get(jt, ()):
                    f()
                if prevE is not None:
                    pv(jt - 1, prevE)
                prevE = E
            pv(ST - 1, prevE)
            attn_norm(hA, ic, OA, direct=direct_norm)
            attn_norm(hB, ic, OB, direct=direct_norm)

        # ---- emission schedule ----
        qk_proj(1, 0, 0)
        qk_proj(0, 0, 0, alt=True)
        v_proj(0)
        v_proj(1)
        F = lambda *fs: list(fs)
        p0i0 = {jt: F(lambda st=jt + 2: v_proj(st)) for jt in range(ST - 2)}
        for jt, c in ((2, 1), (5, 2), (9, 3)):
            p0i0[jt] = [lambda c=c: qk_proj(1, 0, c)] + p0i0[jt]
        p0i0[12] = [lambda: qk_proj(0, 0, 1)] + p0i0[12]
        pair_ic(0, 0, p0i0)
        pair_ic(0, 1, {
            4: F(lambda: qk_proj(1, 1, 0)),
            6: F(lambda: qk_proj(1, 1, 1)),
            8: F(lambda: qk_proj(1, 1, 2)),
            10: F(lambda: qk_proj(1, 1, 3)),
            13: F(lambda: qk_proj(0, 1, 0)),
        })
        pair_ic(1, 0, {
            4: F(lambda: qk_proj(0, 1, 1)),
            9: F(lambda: qk_proj(0, 0, 2)),
        })
        pair_ic(1, 1, {
            4: F(lambda: qk_proj(0, 1, 2)),
            9: F(lambda: qk_proj(0, 0, 3)),
            12: F(lambda: qk_proj(0, 1, 3)),
        })
        pair_ic(0, 2, {5 + i: F(lambda st=(i + 2) // 2, oc=i % 2: o_proj_chunk(st, oc))
                       for i in range(6)})
        pair_ic(1, 2, {4 + i: F(lambda st=4 + i // 2, oc=i % 2: o_proj_chunk(st, oc))
                       for i in range(8)})
        pair_ic(0, 3, {4 + i: F(lambda st=8 + i // 2, oc=i % 2: o_proj_chunk(st, oc))
                       for i in range(4)})
        pair_ic(1, 3, {4: F(lambda: o_proj_chunk(0, 0)),
                       8: F(lambda: o_proj_chunk(0, 1))}, direct_norm=True)

        # ---- dense tail: st 10-15 across all 8 PSUM banks, ordered by when
        # each bank frees.  st10/11 (whose aoT inputs are long ready) and the
        # st12 kt2=0 partials run while the final softmax-normalization chain
        # is on Vector/GpSimd; the ps banks are then re-used for st14/15.
        # st13 sits in the two banks the last pair's O accumulators vacate at
        # the norm multiplies.  kt2=1 for st12-15 follows the norm.
        def omm(piece, st, kt2, oc):
            nc.tensor.matmul(
                piece,
                aoT[:, kt2, st * 128 : (st + 1) * 128],
                wo_sb[:, kt2, oc * 512 : (oc + 1) * 512],
                start=(kt2 == 0),
                stop=(kt2 == 1),
            )

        def evict_dma(st, tile_full, engine):
            stg = tlo.tile([128, D], f32, tag="to")
            if engine == "scalar":
                nc.scalar.activation(stg[:], tile_full[:], Act.Copy)
            else:
                nc.vector.tensor_copy(stg[:, 0:512], tile_full[:, 0:512])
                nc.vector.tensor_copy(stg[:, 512:1024], tile_full[:, 512:1024])
            nc.sync.dma_start(out_d.ap()[st * 128 : (st + 1) * 128, :], stg[:])

        tpsA = ps.tile([128, 2 * IC], f32, tag="S", name="tpsA")  # st10
        tpsB = ps.tile([128, 2 * IC], f32, tag="S", name="tpsB")  # st11
        for st, t in ((10, tpsA), (11, tpsB)):
            for oc in range(2):
                for kt2 in range(2):
                    omm(t[:, oc * 512 : (oc + 1) * 512], st, kt2, oc)
        t12a = op_.tile([128, 512], f32, tag="O", name="t12a")  # early bank
        t12b = fp.tile([128, 512], f32, tag="f", name="t12b")
        omm(t12a[:], 12, 0, 0)
        omm(t12b[:], 12, 0, 1)
        evict_dma(10, tpsA, "scalar")
        evict_dma(11, tpsB, "vector")
        tpsC = ps.tile([128, 2 * IC], f32, tag="S", name="tpsC")  # st14
        tpsD = ps.tile([128, 2 * IC], f32, tag="S", name="tpsD")  # st15
        for oc in range(2):
            omm(tpsC[:, oc * 512 : (oc + 1) * 512], 14, 0, oc)
            omm(tpsD[:, oc * 512 : (oc + 1) * 512], 15, 0, oc)
        t13a = op_.tile([128, 512], f32, tag="O", name="t13a")  # O-acc banks
        t13b = op_.tile([128, 512], f32, tag="O", name="t13b")
        omm(t13a[:], 13, 0, 0)
        omm(t13b[:], 13, 0, 1)
        omm(t12a[:], 12, 1, 0)
        omm(t12b[:], 12, 1, 1)
        omm(t13a[:], 13, 1, 0)
        omm(t13b[:], 13, 1, 1)
        for oc in range(2):
            omm(tpsC[:, oc * 512 : (oc + 1) * 512], 14, 1, oc)
            omm(tpsD[:, oc * 512 : (oc + 1) * 512], 15, 1, oc)
        stg12 = tlo.tile([128, D], f32, tag="to")
        nc.vector.tensor_copy(stg12[:, 0:512], t12a[:])
        nc.vector.tensor_copy(stg12[:, 512:1024], t12b[:])
        nc.sync.dma_start(out_d.ap()[12 * 128 : 13 * 128, :], stg12[:])
        stg13 = tlo.tile([128, D], f32, tag="to")
        nc.vector.tensor_copy(stg13[:, 0:512], t13a[:])
        nc.vector.tensor_copy(stg13[:, 512:1024], t13b[:])
        nc.sync.dma_start(out_d.ap()[13 * 128 : 14 * 128, :], stg13[:])
        evict_dma(14, tpsC, "scalar")
        evict_dma(15, tpsD, "scalar")

    nc.compile()
    return nc


def _get_nc():
    global _nc_cache
    if _nc_cache is None:
        _nc_cache = _build_nc()
    return _nc_cache


def _prepare_in_maps(x, W_q, b_q, W_k, b_k, W_v, b_v, W_o, b_o):
    in_maps = []
    for c in range(N_CORES):
        b, g = c // 4, c % 4
        rows = slice(DL * g, DL * g + DL)
        bqk = np.stack(
            [
                b_q[DL * g : DL * g + 128],
                b_q[DL * g + 128 : DL * g + 256],
                b_k[DL * g : DL * g + 128],
                b_k[DL * g + 128 : DL * g + 256],
            ],
            axis=1,
        ).astype(np.float32)
        # x staged as [p, c, kt, 512]: xt[p, ci, kt, s] = x[b].T[kt*128+p,
        # ci*512+s] so each 512-col chunk is one strided DMA.
        xT = np.ascontiguousarray(x[b].T).astype(_BF16)  # [D, S]
        xt = np.ascontiguousarray(
            xT.reshape(KT, 128, NIC, 512).transpose(1, 2, 0, 3)
        )
        def tile_qk(w):  # [k, dl] -> [p, mhalf, kt, 128]
            return np.ascontiguousarray(
                w.reshape(KT, 128, 2, 128).transpose(1, 2, 0, 3)
            )

        wqT = W_q[rows].T.astype(_BF16)
        wkT = W_k[rows].T.astype(_BF16)
        wvT = W_v[rows].T.astype(_BF16)  # [k, dl]
        woT = W_o[:, rows].T.astype(_BF16)  # [dl, o]
        in_maps.append(
            {
                "xt": xt,
                "wq": tile_qk(wqT),
                "wk": tile_qk(wkT),
                "wv": np.ascontiguousarray(
                    wvT.reshape(KT, 128, DL).transpose(1, 0, 2)
                ),
                "wo": np.ascontiguousarray(
                    woT.reshape(2, 128, D).transpose(1, 0, 2)
                ),
                "bqk": np.ascontiguousarray(bqk),
                "bv": np.ascontiguousarray(
                    np.broadcast_to(b_v[rows], (128, DL))
                ).astype(np.float32),
            }
        )
    return in_maps


def _assemble(results, b_o):
    out = np.empty((B, S, D), dtype=np.float32)
    for b in range(B):
        acc = results[4 * b]["out"].astype(np.float32).copy()
        for g in range(1, 4):
            acc += results[4 * b + g]["out"]
        out[b] = acc + b_o[None, :].astype(np.float32)
    return out


def kernel(x, W_q, b_q, W_k, b_k, W_v, b_v, W_o, b_o):
    from concourse.bass_utils import run_bass_kernel_spmd

    x = np.asarray(x, dtype=np.float32)
    nc = _get_nc()
    in_maps = _prepare_in_maps(
        x,
        np.asarray(W_q, np.float32),
        np.asarray(b_q, np.float32),
        np.asarray(W_k, np.float32),
        np.asarray(b_k, np.float32),
        np.asarray(W_v, np.float32),
        np.asarray(b_v, np.float32),
        np.asarray(W_o, np.float32),
        np.asarray(b_o, np.float32),
    )
    res = run_bass_kernel_spmd(nc, in_maps, core_ids=list(range(N_CORES)))
    return _assemble(res.results, np.asarray(b_o, np.float32))
